# revision 1
# baseline (speedup 1.0000x reference)
"""GPTBigCode transformer block (MQA) on 8 trn2 NeuronCores.

Sharding: data-parallel over batch (4) x sequence-split (2) per batch
element. Core c handles batch c//2 and the interleaved token blocks
{2i + c%2 : i in 0..7} (128 tokens each), which balances causal-attention
work across cores and needs NO collectives: K/V (single MQA head) are
recomputed per core from the full per-batch hidden states.

In-kernel dataflow keeps activations feature-on-partition ("T layout")
so matmul chains need no operand transposes; activations are transposed
once per stage on the tensor engine. LN gains/biases are folded into the
following matmul weights on the host; matmul inputs are bf16, all
accumulation/softmax/residual math is f32.
"""

import numpy as np
import ml_dtypes

# ---------------------------------------------------------------------------
# Workaround: this container's walrus build rejects >1 sync-wait on
# CTRL-class (Drain) instructions. Split the Tile tail-drain's waits into
# individual wait-carrying NOPs on the SP engine.
import bass_rust
from concourse.tile import TileContext
from concourse.vector_clock import ScopedClock


def _patched_drain_and_barrier(self, tick_clock, wait_clock):
    nc = self.nc
    drain_inst = nc.sync.drain()
    wait_clock.add_sem_waits(
        drain_inst.ins, ScopedClock({None: tick_clock.global_clock})
    )
    si = drain_inst.ins.sync_info
    waits = list(si.on_wait) if si and si.on_wait else []
    if len(waits) > 1:
        drain_inst.ins.sync_info = bass_rust.SyncInfo(
            on_wait=waits[:1],
            on_update=list(si.on_update) if si.on_update else [],
        )
        for w in waits[1:]:
            n = nc.sync.nop(nofuse=True, hint="split_drain_wait")
            n.ins.sync_info = bass_rust.SyncInfo(on_wait=[w], on_update=[])
    nc.all_engine_barrier()
    assert self.sems is not None
    popped = nc._tile_sem_poison_stack.pop()
    assert popped is self._sem_poison
    nc.clear_and_free_semaphores(list(self.sems.allocated().values()))
    nc.all_engine_barrier()


TileContext._drain_and_barrier = _patched_drain_and_barrier


def _split_excess_waits(nc, max_waits=1):
    """Rewrite every instruction carrying more than `max_waits` sem-waits:
    excess waits move onto same-engine NOPs inserted just before it."""
    all_bbs = [bb for fn in nc.m.functions for bb in fn.blocks]
    for bb in all_bbs:
        insts = list(bb.instructions)
        new_list = []
        changed = False
        for inst in insts:
            si = inst.sync_info
            waits = list(si.on_wait) if si and si.on_wait else []
            if len(waits) > max_waits:
                changed = True
                inst.sync_info = bass_rust.SyncInfo(
                    on_wait=waits[:max_waits],
                    on_update=list(si.on_update) if si.on_update else [],
                )
                for w in waits[max_waits:]:
                    nop_bi = nc.engines[inst.engine].nop(
                        nofuse=True, hint="wsplit"
                    )
                    nop = nop_bi.ins
                    cur = nc.cur_bb.bb
                    cl = list(cur.instructions)
                    assert cl and cl[-1].name == nop.name, "nop not appended last"
                    cur.instructions = cl[:-1]
                    nop.sync_info = bass_rust.SyncInfo(on_wait=[w], on_update=[])
                    new_list.append(nop)
            new_list.append(inst)
        if changed:
            bb.instructions = new_list
# ---------------------------------------------------------------------------

import concourse.bass as bass
import concourse.mybir as mybir
from concourse.bass_utils import run_bass_kernel_spmd
from concourse.masks import make_identity

f32 = mybir.dt.float32
bf16 = mybir.dt.bfloat16
AF = mybir.ActivationFunctionType
ALU = mybir.AluOpType

H = 2048
NH = 16
D = 128
INTER = 8192
S = 2048
B = 4
NQ = 1024          # query tokens per core
QT = NQ // 128     # 8 local q tiles
KT = S // 128      # 16 key tiles
HT = H // 128      # 16 hidden tiles
IT = INTER // 128  # 64
EPS = 1e-5
NEG = -1e30


def _layernorm(nc, pool, x_t, ln_out, eps_t):
    """x_t [128, H] f32 -> ln_out [128, H] (normalized, no gain/bias)."""
    st = pool.tile([128, 4, 6], f32, tag="st", bufs=3, name="st")
    xr = x_t.rearrange("p (g f) -> p g f", g=4)
    for g in range(4):
        nc.vector.bn_stats(out=st[:, g, :], in_=xr[:, g, :])
    mv = pool.tile([128, 2], f32, tag="mv", bufs=3, name="mv")
    nc.vector.bn_aggr(out=mv, in_=st)
    rstd = pool.tile([128, 1], f32, tag="rstd", bufs=3, name="rstd")
    nc.scalar.activation(out=rstd, in_=mv[:, 1:2], func=AF.Sqrt, bias=eps_t)
    nc.vector.reciprocal(out=rstd, in_=rstd)
    nc.vector.tensor_scalar(
        out=ln_out, in0=x_t, scalar1=mv[:, 0:1], scalar2=rstd,
        op0=ALU.subtract, op1=ALU.mult,
    )


def _build_program():
    nc = bass.Bass(trn_type="TRN2")

    x_full = nc.dram_tensor("x_full", [S, H], f32, kind="ExternalInput")
    x_q = nc.dram_tensor("x_q", [NQ, H], f32, kind="ExternalInput")
    wq_d = nc.dram_tensor("wq", [H, H], bf16, kind="ExternalInput")
    wkv_d = nc.dram_tensor("wkv", [H, 256], bf16, kind="ExternalInput")
    wo_d = nc.dram_tensor("wo", [H, H], bf16, kind="ExternalInput")
    wfc_d = nc.dram_tensor("wfc", [H, INTER], bf16, kind="ExternalInput")
    wproj_d = nc.dram_tensor("wproj", [INTER, H], bf16, kind="ExternalInput")
    bq_d = nc.dram_tensor("bq", [H], f32, kind="ExternalInput")
    bkv_d = nc.dram_tensor("bkv", [256], f32, kind="ExternalInput")
    bo_d = nc.dram_tensor("bo", [H], f32, kind="ExternalInput")
    bfc_d = nc.dram_tensor("bfc", [INTER], f32, kind="ExternalInput")
    bproj_d = nc.dram_tensor("bproj", [H], f32, kind="ExternalInput")
    mask_d = nc.dram_tensor("mask", [128, 256], f32, kind="ExternalInput")
    out_d = nc.dram_tensor("out", [NQ, H], f32, kind="ExternalOutput")

    with TileContext(nc) as tc:
        with (
            tc.tile_pool(name="const", bufs=1) as constp,
            tc.tile_pool(name="big", bufs=1) as bigp,
            tc.tile_pool(name="b4", bufs=2) as b4p,
            tc.tile_pool(name="kvp", bufs=1) as kvp,
            tc.tile_pool(name="work", bufs=2) as workp,
            tc.tile_pool(name="psum", bufs=2, space="PSUM") as psump,
            tc.tile_pool(name="dram", bufs=1, space="DRAM") as dramp,
        ):
            # ---- constants ----
            id_f = constp.tile([128, 128], f32, name="id_f")
            make_identity(nc, id_f)
            id_b = constp.tile([128, 128], bf16, name="id_b")
            make_identity(nc, id_b)
            eps_t = constp.tile([128, 1], f32, name="eps_t")
            nc.vector.memset(eps_t, EPS)
            mask_sb = constp.tile([128, 256], f32, name="mask_sb")
            nc.sync.dma_start(mask_sb, mask_d[:, :])
            bq_sb = constp.tile([128, HT], f32, name="bq_sb")
            nc.sync.dma_start(bq_sb, bq_d.rearrange("(m p) -> p m", p=128))
            bkv_sb = constp.tile([128, 2], f32, name="bkv_sb")
            nc.sync.dma_start(bkv_sb, bkv_d.rearrange("(m p) -> p m", p=128))
            bo_sb = constp.tile([128, HT], f32, name="bo_sb")
            nc.sync.dma_start(bo_sb, bo_d.rearrange("(m p) -> p m", p=128))
            bfc_sb = constp.tile([128, IT], f32, name="bfc_sb")
            nc.sync.dma_start(bfc_sb, bfc_d.rearrange("(m p) -> p m", p=128))
            bproj_sb = constp.tile([128, HT], f32, name="bproj_sb")
            nc.sync.dma_start(bproj_sb, bproj_d.rearrange("(m p) -> p m", p=128))
            wkv_sb = constp.tile([128, HT, 256], bf16, name="wkv_sb")
            nc.sync.dma_start(wkv_sb, wkv_d.rearrange("(k p) n -> p k n", p=128))

            y_dram = dramp.tile([NQ, H], f32, name="y_dram")

            wq_r = wq_d.rearrange("(k p) n -> p k n", p=128)
            wo_r = wo_d.rearrange("(k p) n -> p k n", p=128)
            wfc_r = wfc_d.rearrange("(k p) n -> p k n", p=128)
            wproj_r = wproj_d.rearrange("(k p) n -> p k n", p=128)

            # ---- S1: LN1 over full sequence, transposed -> lnT ----
            lnT = bigp.tile([128, HT, S], bf16, tag="big8", name="lnT")
            for t in range(KT):
                x_t = workp.tile([128, H], f32, tag="xf", bufs=2, name="x_t")
                nc.sync.dma_start(x_t, x_full[t * 128:(t + 1) * 128, :])
                ln_t = workp.tile([128, H], bf16, tag="lnb", bufs=2, name="ln_t")
                _layernorm(nc, workp, x_t, ln_t, eps_t)
                for k in range(HT):
                    pt = psump.tile([128, 128], bf16, tag="tp", bufs=4, name="pt")
                    nc.tensor.transpose(pt, ln_t[:, k * 128:(k + 1) * 128], id_b)
                    nc.scalar.copy(lnT[:, k, t * 128:(t + 1) * 128], pt)

            # ---- S2: K/V heads (K pre-scaled on host) ----
            kT_sb = kvp.tile([128, S], bf16, name="kT_sb")
            vT_sb = workp.tile([128, S], bf16, tag="h", bufs=2, name="vT_sb")
            for m in range(2):
                for n4 in range(4):
                    pk = psump.tile([128, 512], f32, tag="sc", bufs=2, name="pk")
                    for k in range(HT):
                        nc.tensor.matmul(
                            pk, wkv_sb[:, k, m * 128:(m + 1) * 128],
                            lnT[:, k, n4 * 512:(n4 + 1) * 512],
                            start=(k == 0), stop=(k == HT - 1),
                        )
                    dst = kT_sb if m == 0 else vT_sb
                    nc.scalar.activation(
                        dst[:, n4 * 512:(n4 + 1) * 512], pk, AF.Identity,
                        bias=bkv_sb[:, m:m + 1],
                    )
            v_sb = kvp.tile([128, KT, 128], bf16, name="v_sb")
            for j in range(KT):
                pt = psump.tile([128, 128], bf16, tag="tp", bufs=4, name="pt")
                nc.tensor.transpose(pt, vT_sb[:, j * 128:(j + 1) * 128], id_b)
                nc.scalar.copy(v_sb[:, j, :], pt)

            # ---- S3: LN1 of own query tokens, transposed -> lnqT ----
            lnqT = b4p.tile([128, HT, NQ], bf16, tag="b4", name="lnqT")
            for t in range(QT):
                xq_t = workp.tile([128, H], f32, tag="xf", bufs=2, name="xq_t")
                nc.sync.dma_start(xq_t, x_q[t * 128:(t + 1) * 128, :])
                lnq_t = workp.tile([128, H], bf16, tag="lnb", bufs=2, name="lnq_t")
                _layernorm(nc, workp, xq_t, lnq_t, eps_t)
                for k in range(HT):
                    pt = psump.tile([128, 128], bf16, tag="tp", bufs=4, name="pt")
                    nc.tensor.transpose(pt, lnq_t[:, k * 128:(k + 1) * 128], id_b)
                    nc.scalar.copy(lnqT[:, k, t * 128:(t + 1) * 128], pt)

            # ---- S4: qT = wq^T @ lnqT (+bq) ----
            qT = b4p.tile([128, NH, NQ], bf16, tag="b4", name="qT")
            for m in range(HT):
                band = workp.tile([128, HT, 128], bf16, tag="band", bufs=3, name="band")
                nc.sync.dma_start(band, wq_r[:, :, m * 128:(m + 1) * 128])
                for half in range(2):
                    pq = psump.tile([128, 512], f32, tag="sc", bufs=2, name="pq")
                    for k in range(HT):
                        nc.tensor.matmul(
                            pq, band[:, k, :],
                            lnqT[:, k, half * 512:(half + 1) * 512],
                            start=(k == 0), stop=(k == HT - 1),
                        )
                    nc.scalar.activation(
                        qT[:, m, half * 512:(half + 1) * 512], pq, AF.Identity,
                        bias=bq_sb[:, m:m + 1],
                    )

            # ---- S5: attention (causal, static key-extent per q tile) ----
            attnT = b4p.tile([128, NH, NQ], bf16, tag="b4", name="attnT")
            for hd in range(NH):
                for i in range(QT):
                    nk = (2 * i + 2) * 128
                    nch = (nk + 511) // 512
                    probs = workp.tile([128, 2048], bf16, tag="h", bufs=2, name="probs")
                    den = workp.tile([128, 4], f32, tag="den", bufs=3, name="den")
                    for ch in range(nch):
                        w = min(512, nk - ch * 512)
                        ps = psump.tile([128, 512], f32, tag="sc", bufs=2, name="ps")
                        nc.tensor.matmul(
                            ps[:, :w], qT[:, hd, i * 128:(i + 1) * 128],
                            kT_sb[:, ch * 512:ch * 512 + w],
                            start=True, stop=True,
                        )
                        if ch == nch - 1:
                            nc.vector.tensor_add(
                                ps[:, w - 256:w], ps[:, w - 256:w], mask_sb
                            )
                        nc.scalar.activation(
                            probs[:, ch * 512:ch * 512 + w], ps[:, :w], AF.Exp,
                            accum_out=den[:, ch:ch + 1],
                        )
                    rec = workp.tile([128, 1], f32, tag="rec", bufs=3, name="rec")
                    if nch > 1:
                        nc.vector.reduce_sum(rec, den[:, :nch], axis=mybir.AxisListType.X)
                        nc.vector.reciprocal(rec, rec)
                    else:
                        nc.vector.reciprocal(rec, den[:, 0:1])
                    pa = psump.tile([128, 128], f32, tag="acc", bufs=2, name="pa")
                    nkt = 2 * i + 2
                    for kt in range(nkt):
                        ptp = psump.tile([128, 128], bf16, tag="tp", bufs=4, name="ptp")
                        nc.tensor.transpose(
                            ptp, probs[:, kt * 128:(kt + 1) * 128], id_b
                        )
                        pT = workp.tile([128, 128], bf16, tag="pT", bufs=4, name="pT")
                        if kt % 2:
                            nc.vector.tensor_copy(pT, ptp)
                        else:
                            nc.scalar.copy(pT, ptp)
                        nc.tensor.matmul(
                            pa, pT, v_sb[:, kt, :],
                            start=(kt == 0), stop=(kt == nkt - 1),
                        )
                    att = workp.tile([128, 128], bf16, tag="att", bufs=3, name="att")
                    nc.vector.tensor_scalar_mul(att, pa, rec)
                    pat = psump.tile([128, 128], bf16, tag="tp", bufs=4, name="pat")
                    nc.tensor.transpose(pat, att, id_b)
                    nc.scalar.copy(attnT[:, hd, i * 128:(i + 1) * 128], pat)

            # ---- S6: y = attn @ wo + bo + x_q  -> y_dram ----
            for m in range(HT):
                band = workp.tile([128, NH, 128], bf16, tag="band", bufs=3, name="band")
                nc.sync.dma_start(band, wo_r[:, :, m * 128:(m + 1) * 128])
                yTb = workp.tile([128, NQ], f32, tag="yT", bufs=2, name="yTb")
                for half in range(2):
                    py = psump.tile([128, 512], f32, tag="sc", bufs=2, name="py")
                    for k in range(NH):
                        nc.tensor.matmul(
                            py, band[:, k, :],
                            attnT[:, k, half * 512:(half + 1) * 512],
                            start=(k == 0), stop=(k == NH - 1),
                        )
                    nc.scalar.activation(
                        yTb[:, half * 512:(half + 1) * 512], py, AF.Identity,
                        bias=bo_sb[:, m:m + 1],
                    )
                for t in range(QT):
                    ptp = psump.tile([128, 128], f32, tag="tp", bufs=4, name="ptp")
                    nc.tensor.transpose(ptp, yTb[:, t * 128:(t + 1) * 128], id_f)
                    rb = workp.tile([128, 128], f32, tag="xqb", bufs=4, name="rb")
                    nc.sync.dma_start(
                        rb, x_q[t * 128:(t + 1) * 128, m * 128:(m + 1) * 128]
                    )
                    yb = workp.tile([128, 128], f32, tag="yb", bufs=4, name="yb")
                    nc.vector.tensor_add(yb, ptp, rb)
                    nc.sync.dma_start(
                        y_dram[t * 128:(t + 1) * 128, m * 128:(m + 1) * 128], yb
                    )

            # ---- S7: LN2, transposed -> ln2T ----
            ln2T = b4p.tile([128, HT, NQ], bf16, tag="b4", name="ln2T")
            for t in range(QT):
                y_t = workp.tile([128, H], f32, tag="xf", bufs=2, name="y_t")
                nc.sync.dma_start(y_t, y_dram[t * 128:(t + 1) * 128, :])
                ln2_t = workp.tile([128, H], bf16, tag="lnb", bufs=2, name="ln2_t")
                _layernorm(nc, workp, y_t, ln2_t, eps_t)
                for k in range(HT):
                    pt = psump.tile([128, 128], bf16, tag="tp", bufs=4, name="pt")
                    nc.tensor.transpose(pt, ln2_t[:, k * 128:(k + 1) * 128], id_b)
                    nc.scalar.copy(ln2T[:, k, t * 128:(t + 1) * 128], pt)

            # ---- S8/S9: MLP in two token halves (gT fits one 8MB slot) ----
            for hq in range(2):
                tok0 = hq * 512
                gT = bigp.tile([128, IT, 512], bf16, tag="big8", name="gT")
                for mi in range(IT):
                    band = workp.tile([128, HT, 128], bf16, tag="band", bufs=3, name="band")
                    nc.sync.dma_start(band, wfc_r[:, :, mi * 128:(mi + 1) * 128])
                    pf = psump.tile([128, 512], f32, tag="sc", bufs=2, name="pf")
                    for k in range(HT):
                        nc.tensor.matmul(
                            pf, band[:, k, :], ln2T[:, k, tok0:tok0 + 512],
                            start=(k == 0), stop=(k == HT - 1),
                        )
                    nc.scalar.activation(
                        gT[:, mi, :], pf, AF.Gelu_apprx_tanh,
                        bias=bfc_sb[:, mi:mi + 1],
                    )
                for m in range(HT):
                    po = psump.tile([128, 512], f32, tag="sc", bufs=2, name="po")
                    for kg in range(4):
                        band = workp.tile(
                            [128, HT, 128], bf16, tag="band", bufs=3, name="band"
                        )
                        nc.sync.dma_start(
                            band, wproj_r[:, kg * 16:(kg + 1) * 16, m * 128:(m + 1) * 128]
                        )
                        for kk in range(HT):
                            k = kg * 16 + kk
                            nc.tensor.matmul(
                                po, band[:, kk, :], gT[:, k, :],
                                start=(k == 0), stop=(k == IT - 1),
                            )
                    oT = workp.tile([128, 512], f32, tag="yT", bufs=2, name="oT")
                    nc.scalar.activation(
                        oT, po, AF.Identity, bias=bproj_sb[:, m:m + 1]
                    )
                    for tt in range(4):
                        t = hq * 4 + tt
                        ptp = psump.tile([128, 128], f32, tag="tp", bufs=4, name="ptp")
                        nc.tensor.transpose(ptp, oT[:, tt * 128:(tt + 1) * 128], id_f)
                        rb = workp.tile([128, 128], f32, tag="xqb", bufs=4, name="rb")
                        nc.sync.dma_start(
                            rb, y_dram[t * 128:(t + 1) * 128, m * 128:(m + 1) * 128]
                        )
                        yb = workp.tile([128, 128], f32, tag="yb", bufs=4, name="yb")
                        nc.vector.tensor_add(yb, ptp, rb)
                        nc.sync.dma_start(
                            out_d[t * 128:(t + 1) * 128, m * 128:(m + 1) * 128], yb
                        )
    _split_excess_waits(nc)
    return nc


_PROG = None


def _get_prog():
    global _PROG
    if _PROG is None:
        _PROG = _build_program()
    return _PROG


def kernel(hidden_states, ln1_g, ln1_b, ln2_g, ln2_b, wq, bq, wkv, bkv,
           wo, bo, wfc, bfc, wproj, bproj):
    hs = np.asarray(hidden_states, np.float32)
    ln1_g = np.asarray(ln1_g, np.float32)
    ln1_b = np.asarray(ln1_b, np.float32)
    ln2_g = np.asarray(ln2_g, np.float32)
    ln2_b = np.asarray(ln2_b, np.float32)
    wq = np.asarray(wq, np.float32)
    wkv = np.asarray(wkv, np.float32)
    wo = np.asarray(wo, np.float32)
    wfc = np.asarray(wfc, np.float32)
    wproj = np.asarray(wproj, np.float32)

    # Fold LN gain/bias into the following matmuls; fold qk scale into K.
    wq_e = ln1_g[:, None] * wq
    bq_e = np.asarray(bq, np.float32) + ln1_b @ wq
    wkv_e = (ln1_g[:, None] * wkv).copy()
    bkv_e = (np.asarray(bkv, np.float32) + ln1_b @ wkv).copy()
    scale = 1.0 / np.sqrt(D)
    wkv_e[:, :D] *= scale
    bkv_e[:D] *= scale
    wfc_e = ln2_g[:, None] * wfc
    bfc_e = np.asarray(bfc, np.float32) + ln2_b @ wfc

    def to_bf(a):
        return np.ascontiguousarray(a.astype(ml_dtypes.bfloat16))

    wq_b, wkv_b, wo_b = to_bf(wq_e), to_bf(wkv_e), to_bf(wo)
    wfc_b, wproj_b = to_bf(wfc_e), to_bf(wproj)
    bo_f = np.ascontiguousarray(np.asarray(bo, np.float32))
    bproj_f = np.ascontiguousarray(np.asarray(bproj, np.float32))
    bq_e = np.ascontiguousarray(bq_e)
    bfc_e = np.ascontiguousarray(bfc_e)

    tril = np.where(np.tril(np.ones((128, 128), bool)), 0.0, NEG).astype(np.float32)
    mask_h = [
        np.ascontiguousarray(
            np.concatenate([tril, np.full((128, 128), NEG, np.float32)], axis=1)),
        np.ascontiguousarray(
            np.concatenate([np.zeros((128, 128), np.float32), tril], axis=1)),
    ]

    in_maps = []
    for c in range(8):
        b, h = divmod(c, 2)
        xb = np.ascontiguousarray(hs[b])
        xq = np.ascontiguousarray(xb.reshape(8, 2, 128, H)[:, h].reshape(NQ, H))
        in_maps.append(dict(
            x_full=xb, x_q=xq, wq=wq_b, wkv=wkv_b, wo=wo_b, wfc=wfc_b,
            wproj=wproj_b, bq=bq_e, bkv=bkv_e, bo=bo_f, bfc=bfc_e,
            bproj=bproj_f, mask=mask_h[h],
        ))

    global last_in_maps
    last_in_maps = in_maps
    res = run_bass_kernel_spmd(_get_prog(), in_maps, core_ids=list(range(8)))
    kernel.last_result = res

    out = np.empty((B, S, H), np.float32)
    for c in range(8):
        b, h = divmod(c, 2)
        out[b].reshape(8, 2, 128, H)[:, h] = (
            np.asarray(res.results[c]["out"]).reshape(8, 128, H)
        )
    return out



# revision 8
# speedup vs baseline: 1.1101x; 1.1101x over previous
"""GPTBigCode transformer block (MQA) on 8 trn2 NeuronCores.

Sharding: data-parallel over batch (4) x sequence-split (2) per batch
element; core c handles batch c//2 and the interleaved global token
blocks {2i + c%2}. No collectives: the single MQA K/V head is
recomputed per core over the full (permuted) sequence.

Dataflow keeps every activation feature-on-partition ("T layout") from
input to output, so there are NO on-chip transposes:
  - x arrives pre-transposed (host), with columns permuted so the
    core's own 1024 tokens come first (makes the program h-independent).
  - LN stats = column sums via ones-vector matmuls; mean/rstd are
    re-broadcast across partitions with K=1 matmuls.
  - attention runs in scores-transposed layout [keys, queries]; the
    causal mask is one fused DVE op (qg >= k_idx) * exp; softmax
    denominators come from ones-matmuls; V is produced in natural
    layout so probs feed the AV matmul directly.
  - weights are host-pretiled so every weight DMA is contiguous.
LN gains/biases and the 1/sqrt(D) score scale are folded into weights
on the host; matmul inputs are bf16, accumulation/softmax f32.
"""

import numpy as np
import ml_dtypes

# ---------------------------------------------------------------------------
# Workaround: this container's walrus build rejects >1 sync-wait on
# CTRL-class (Drain) instructions. Split the Tile tail-drain's waits into
# individual wait-carrying NOPs on the SP engine.
import bass_rust
from concourse.tile import TileContext
from concourse.vector_clock import ScopedClock


def _patched_drain_and_barrier(self, tick_clock, wait_clock):
    nc = self.nc
    drain_inst = nc.sync.drain()
    wait_clock.add_sem_waits(
        drain_inst.ins, ScopedClock({None: tick_clock.global_clock})
    )
    si = drain_inst.ins.sync_info
    waits = list(si.on_wait) if si and si.on_wait else []
    if len(waits) > 1:
        drain_inst.ins.sync_info = bass_rust.SyncInfo(
            on_wait=waits[:1],
            on_update=list(si.on_update) if si.on_update else [],
        )
        for w in waits[1:]:
            n = nc.sync.nop(nofuse=True, hint="split_drain_wait")
            n.ins.sync_info = bass_rust.SyncInfo(on_wait=[w], on_update=[])
    nc.all_engine_barrier()
    assert self.sems is not None
    popped = nc._tile_sem_poison_stack.pop()
    assert popped is self._sem_poison
    nc.clear_and_free_semaphores(list(self.sems.allocated().values()))
    nc.all_engine_barrier()


TileContext._drain_and_barrier = _patched_drain_and_barrier


def _split_excess_waits(nc, max_waits=1):
    """Rewrite every instruction carrying more than `max_waits` sem-waits:
    excess waits move onto same-engine NOPs inserted just before it."""
    all_bbs = [bb for fn in nc.m.functions for bb in fn.blocks]
    for bb in all_bbs:
        insts = list(bb.instructions)
        new_list = []
        changed = False
        for inst in insts:
            si = inst.sync_info
            waits = list(si.on_wait) if si and si.on_wait else []
            if len(waits) > max_waits:
                changed = True
                inst.sync_info = bass_rust.SyncInfo(
                    on_wait=waits[:max_waits],
                    on_update=list(si.on_update) if si.on_update else [],
                )
                for w in waits[max_waits:]:
                    nop_bi = nc.engines[inst.engine].nop(
                        nofuse=True, hint="wsplit"
                    )
                    nop = nop_bi.ins
                    cur = nc.cur_bb.bb
                    cl = list(cur.instructions)
                    assert cl and cl[-1].name == nop.name, "nop not appended last"
                    cur.instructions = cl[:-1]
                    nop.sync_info = bass_rust.SyncInfo(on_wait=[w], on_update=[])
                    new_list.append(nop)
            new_list.append(inst)
        if changed:
            bb.instructions = new_list
# ---------------------------------------------------------------------------

import concourse.bass as bass
import concourse.mybir as mybir
from concourse.bass_utils import run_bass_kernel_spmd

f32 = mybir.dt.float32
bf16 = mybir.dt.bfloat16
AF = mybir.ActivationFunctionType
ALU = mybir.AluOpType

H = 2048
NH = 16
D = 128
INTER = 8192
S = 2048
B = 4
NQ = 1024          # query tokens per core
HT = H // 128      # 16
IT = INTER // 128  # 64
KT = S // 128      # 16
EPS = 1e-5


def _ln_stats_smalls(nc, srowp, bcp, mmp, ones1_bf, eps_t, sum_ap, sqsum_ap, inv_n):
    """From psum column-sums (sum_ap, sqsum_ap: [1,512]) produce
    broadcast SBUF bf16 tiles (mb, rb) [128, 512]: mean and rstd."""
    m_row = srowp.tile([1, 512], f32, tag="srow", bufs=3, name="m_row")
    nc.vector.tensor_scalar_mul(m_row, sum_ap, inv_n)
    msq = srowp.tile([1, 512], f32, tag="srow", bufs=3, name="msq")
    nc.vector.tensor_mul(msq, m_row, m_row)
    var_row = srowp.tile([1, 512], f32, tag="srow", bufs=3, name="var_row")
    # var = sqsum/n - mean^2
    nc.vector.scalar_tensor_tensor(
        out=var_row, in0=sqsum_ap, scalar=inv_n, in1=msq,
        op0=ALU.mult, op1=ALU.subtract,
    )
    sd = srowp.tile([1, 512], f32, tag="srow", bufs=3, name="sd")
    nc.scalar.activation(sd, var_row, AF.Sqrt, bias=eps_t)
    r_row = srowp.tile([1, 512], f32, tag="srow", bufs=3, name="r_row")
    nc.vector.reciprocal(r_row, sd)
    m_bf = srowp.tile([1, 512], bf16, tag="srowb", bufs=3, name="m_bf")
    nc.vector.tensor_copy(m_bf, m_row)
    r_bf = srowp.tile([1, 512], bf16, tag="srowb", bufs=3, name="r_bf")
    nc.vector.tensor_copy(r_bf, r_row)
    mb_ps = mmp.tile([128, 512], f32, tag="mm", bufs=2, name="mb_ps")
    nc.tensor.matmul(mb_ps, ones1_bf, m_bf, start=True, stop=True)
    mb = bcp.tile([128, 512], bf16, tag="bc", bufs=8, name="mb")
    nc.vector.tensor_copy(mb, mb_ps)
    rb_ps = mmp.tile([128, 512], f32, tag="mm", bufs=2, name="rb_ps")
    nc.tensor.matmul(rb_ps, ones1_bf, r_bf, start=True, stop=True)
    rb = bcp.tile([128, 512], bf16, tag="bc", bufs=8, name="rb")
    nc.vector.tensor_copy(rb, rb_ps)
    return mb, rb


def _build_program():
    nc = bass.Bass(trn_type="TRN2")

    xT_d = nc.dram_tensor("xT", [H, S], bf16, kind="ExternalInput")
    xqT_d = nc.dram_tensor("xqT", [H, NQ], f32, kind="ExternalInput")
    wq_d = nc.dram_tensor("wq", [H, H], bf16, kind="ExternalInput")
    wo_d = nc.dram_tensor("wo", [H, H], bf16, kind="ExternalInput")
    wfc_d = nc.dram_tensor("wfc", [INTER, H], bf16, kind="ExternalInput")
    wproj_d = nc.dram_tensor("wproj", [H, INTER], bf16, kind="ExternalInput")
    wkvK_d = nc.dram_tensor("wkvK", [128, H], bf16, kind="ExternalInput")
    wkvV_d = nc.dram_tensor("wkvV", [128, H], bf16, kind="ExternalInput")
    bq_d = nc.dram_tensor("bq", [H], f32, kind="ExternalInput")
    bo_d = nc.dram_tensor("bo", [H], f32, kind="ExternalInput")
    bfc_d = nc.dram_tensor("bfc", [INTER], f32, kind="ExternalInput")
    bproj_d = nc.dram_tensor("bproj", [H], f32, kind="ExternalInput")
    bkK_d = nc.dram_tensor("bkK", [128], f32, kind="ExternalInput")
    bkV_d = nc.dram_tensor("bkV", [128], bf16, kind="ExternalInput")
    qgb_d = nc.dram_tensor("qgb", [2 * 128, 512], f32, kind="ExternalInput")
    kvec_d = nc.dram_tensor("kvec", [128, KT], f32, kind="ExternalInput")
    out_d = nc.dram_tensor("outT", [H, NQ], f32, kind="ExternalOutput")

    with TileContext(nc) as tc:
        with (
            tc.tile_pool(name="const", bufs=1) as constp,
            tc.tile_pool(name="big", bufs=1) as bigp,        # 64KB/p slot ring
            tc.tile_pool(name="b32", bufs=2) as b32p,        # 32KB/p slots
            tc.tile_pool(name="kv", bufs=1) as kvp,
            tc.tile_pool(name="band", bufs=3) as bandp,
            tc.tile_pool(name="row", bufs=4) as rowp,
            tc.tile_pool(name="work", bufs=2) as workp,
            tc.tile_pool(name="srow", bufs=1) as srowp,
            tc.tile_pool(name="bc", bufs=1) as bcp,
            tc.tile_pool(name="mm", bufs=2, space="PSUM") as mmp,
            tc.tile_pool(name="acc", bufs=4, space="PSUM") as accp,
            tc.tile_pool(name="aux", bufs=2, space="PSUM") as auxp,
            tc.tile_pool(name="dram", bufs=1, space="DRAM") as dramp,
        ):
            # ---- constants ----
            ones_bf = constp.tile([128, 1], bf16, name="ones_bf")
            nc.vector.memset(ones_bf, 1.0)
            ones1_bf = constp.tile([1, 128], bf16, name="ones1_bf")
            nc.vector.memset(ones1_bf, 1.0)
            ones_f = constp.tile([128, 1], f32, name="ones_f")
            nc.vector.memset(ones_f, 1.0)
            eps_t = constp.tile([1, 1], f32, name="eps_t")
            nc.vector.memset(eps_t, EPS)
            wkvK_sb = constp.tile([128, HT, 128], bf16, name="wkvK_sb")
            nc.sync.dma_start(wkvK_sb, wkvK_d.rearrange("p (k n) -> p k n", n=128))
            wkvV_sb = constp.tile([128, HT, 128], bf16, name="wkvV_sb")
            nc.sync.dma_start(wkvV_sb, wkvV_d.rearrange("p (k n) -> p k n", n=128))
            bq_sb = constp.tile([128, HT], f32, name="bq_sb")
            nc.sync.dma_start(bq_sb, bq_d.rearrange("(m p) -> p m", p=128))
            bo_sb = constp.tile([128, HT], f32, name="bo_sb")
            nc.sync.dma_start(bo_sb, bo_d.rearrange("(m p) -> p m", p=128))
            bfc_sb = constp.tile([128, IT], f32, name="bfc_sb")
            nc.sync.dma_start(bfc_sb, bfc_d.rearrange("(m p) -> p m", p=128))
            bproj_sb = constp.tile([128, HT], f32, name="bproj_sb")
            nc.sync.dma_start(bproj_sb, bproj_d.rearrange("(m p) -> p m", p=128))
            bkK_sb = constp.tile([128, 1], f32, name="bkK_sb")
            nc.sync.dma_start(bkK_sb, bkK_d.rearrange("(m p) -> p m", p=128))
            bkV_sb = constp.tile([1, 128], bf16, name="bkV_sb")
            nc.sync.dma_start(bkV_sb, bkV_d.rearrange("(m p) -> p m", p=1))
            qgb_sb = constp.tile([128, 2, 512], f32, name="qgb_sb")
            nc.sync.dma_start(qgb_sb, qgb_d.rearrange("(j p) n -> p j n", p=128))
            kvec_sb = constp.tile([128, KT], f32, name="kvec_sb")
            nc.sync.dma_start(kvec_sb, kvec_d[:, :])

            ydram = dramp.tile([H, NQ], f32, name="ydram")

            wq_r = wq_d.rearrange("(m p) (k n) -> m p k n", p=128, n=128)
            wo_r = wo_d.rearrange("(m p) (k n) -> m p k n", p=128, n=128)
            wfc_r = wfc_d.rearrange("(m p) (k n) -> m p k n", p=128, n=128)
            wproj_r = wproj_d.rearrange("(m p) (k n) -> m p k n", p=128, n=128)

            # ---- LN1 stats: column sums of x and x^2 over all 16 k tiles ----
            with nc.named_scope("ln1"):
                mean_bank = auxp.tile([128, 512], f32, tag="aux", name="mean_bank")
                var_bank = auxp.tile([128, 512], f32, tag="aux", name="var_bank")
                for k in range(KT):
                    xk = workp.tile([128, S], bf16, tag="xk", bufs=2, name="xk")
                    nc.sync.dma_start(xk, xT_d[k * 128:(k + 1) * 128, :])
                    for g in range(4):
                        xg = xk[:, g * 512:(g + 1) * 512]
                        sq = workp.tile([128, 512], bf16, tag="sq", bufs=3, name="sq")
                        nc.scalar.activation(sq, xg, AF.Square)
                        first = (k == 0 and g == 0)
                        last = (k == KT - 1 and g == 3)
                        nc.tensor.matmul(
                            mean_bank[32 * g:32 * g + 1, :], ones_bf, xg,
                            start=first, stop=last, skip_group_check=True,
                            tile_position=(0, 32 * g),
                        )
                        nc.tensor.matmul(
                            var_bank[32 * g:32 * g + 1, :], ones_bf, sq,
                            start=first, stop=last, skip_group_check=True,
                            tile_position=(0, 32 * g),
                        )
                mbs, rbs = [], []
                for g in range(4):
                    mb, rb = _ln_stats_smalls(
                        nc, srowp, bcp, mmp, ones1_bf, eps_t,
                        mean_bank[32 * g:32 * g + 1, :],
                        var_bank[32 * g:32 * g + 1, :], 1.0 / H,
                    )
                    mbs.append(mb)
                    rbs.append(rb)

                # ---- LN1 apply -> lnT (bf16, feature-major, permuted cols) ----
                lnT = bigp.tile([128, KT, S], bf16, tag="t64", name="lnT")
                for k in range(KT):
                    xk2 = workp.tile([128, S], bf16, tag="xk", bufs=2, name="xk2")
                    nc.sync.dma_start(xk2, xT_d[k * 128:(k + 1) * 128, :])
                    for g in range(4):
                        t1 = workp.tile([128, 512], bf16, tag="t1", bufs=3, name="t1")
                        nc.vector.tensor_sub(t1, xk2[:, g * 512:(g + 1) * 512], mbs[g])
                        nc.vector.tensor_mul(lnT[:, k, g * 512:(g + 1) * 512], t1, rbs[g])

            # ---- K (T layout, pre-scaled) and V (natural layout) ----
            with nc.named_scope("kv"):
                kT_sb = kvp.tile([128, S], bf16, name="kT_sb")
                for g in range(4):
                    kp = accp.tile([128, 512], f32, tag="acc", name="kp")
                    for k in range(KT):
                        nc.tensor.matmul(
                            kp, wkvK_sb[:, k, :], lnT[:, k, g * 512:(g + 1) * 512],
                            start=(k == 0), stop=(k == KT - 1),
                        )
                    nc.scalar.activation(
                        kT_sb[:, g * 512:(g + 1) * 512], kp, AF.Identity, bias=bkK_sb
                    )
                v_sb = kvp.tile([128, KT, 128], bf16, name="v_sb")
                for tt in range(KT):
                    vp = mmp.tile([128, 512], f32, tag="mm", name="vp")
                    vps = vp[:, 0:128]
                    for k in range(KT):
                        nc.tensor.matmul(
                            vps, lnT[:, k, tt * 128:(tt + 1) * 128], wkvV_sb[:, k, :],
                            start=(k == 0), stop=False,
                        )
                    nc.tensor.matmul(vps, ones1_bf, bkV_sb, start=False, stop=True)
                    nc.scalar.copy(v_sb[:, tt, :], vps)

            # ---- Q = wq^T @ lnT[:, own 1024 cols] (+bq) ----
            with nc.named_scope("q"):
                qT = b32p.tile([128, NH, NQ], bf16, tag="t32", name="qT")
                for m in range(HT):
                    band = bandp.tile([128, HT, 128], bf16, tag="band", bufs=3, name="band")
                    nc.sync.dma_start(band, wq_r[m])
                    for half in range(2):
                        qp = accp.tile([128, 512], f32, tag="acc", name="qp")
                        for k in range(KT):
                            nc.tensor.matmul(
                                qp, band[:, k, :],
                                lnT[:, k, half * 512:(half + 1) * 512],
                                start=(k == 0), stop=(k == KT - 1),
                            )
                        nc.scalar.activation(
                            qT[:, m, half * 512:(half + 1) * 512], qp, AF.Identity,
                            bias=bq_sb[:, m:m + 1],
                        )

            # ---- attention: scoresT layout [keys, queries], MQA ----
            with nc.named_scope("attn"):
                attnT = b32p.tile([128, NH, NQ], bf16, tag="t32", name="attnT")
                for hd in range(NH):
                    for j in range(2):
                        kts = list(range(0, 4 * j + 4)) + list(range(8, 8 + 4 * j + 4))
                        den = auxp.tile([1, 512], f32, tag="aux", name="den")
                        av = accp.tile([128, 512], f32, tag="acc", name="av")
                        qmv = qT[:, hd, j * 512:(j + 1) * 512]
                        for idx, kt in enumerate(kts):
                            first, last = (idx == 0), (idx == len(kts) - 1)
                            sc = mmp.tile([128, 512], f32, tag="mm", name="sc")
                            nc.tensor.matmul(
                                sc, kT_sb[:, kt * 128:(kt + 1) * 128], qmv,
                                start=True, stop=True,
                            )
                            ex = workp.tile([128, 512], bf16, tag="ex", bufs=3, name="ex")
                            nc.scalar.activation(ex, sc, AF.Exp)
                            # causal mask: ex = (q_global_idx >= k_global_idx) * ex
                            nc.vector.scalar_tensor_tensor(
                                out=ex, in0=qgb_sb[:, j, :],
                                scalar=kvec_sb[:, kt:kt + 1], in1=ex,
                                op0=ALU.is_ge, op1=ALU.mult,
                            )
                            nc.tensor.matmul(den, ones_bf, ex, start=first, stop=last)
                            nc.tensor.matmul(av, v_sb[:, kt, :], ex, start=first, stop=last)
                        rec = srowp.tile([1, 512], f32, tag="srow", bufs=3, name="rec")
                        nc.vector.reciprocal(rec, den)
                        rec_bf = srowp.tile([1, 512], bf16, tag="srowb", bufs=3, name="rec_bf")
                        nc.vector.tensor_copy(rec_bf, rec)
                        bc = mmp.tile([128, 512], f32, tag="mm", name="bcr")
                        nc.tensor.matmul(bc, ones1_bf, rec_bf, start=True, stop=True)
                        bcs = workp.tile([128, 512], bf16, tag="ex", bufs=3, name="bcs")
                        nc.vector.tensor_copy(bcs, bc)
                        nc.vector.tensor_mul(
                            attnT[:, hd, j * 512:(j + 1) * 512], av, bcs
                        )

            # ---- y = attn @ wo + bo + xq  (T layout) + LN2 stats on the fly ----
            with nc.named_scope("wo"):
                stat2 = auxp.tile([128, 512], f32, tag="aux", name="stat2")
                for m in range(HT):
                    band = bandp.tile([128, NH, 128], bf16, tag="band", bufs=3, name="bando")
                    nc.sync.dma_start(band, wo_r[m])
                    yrow = rowp.tile([128, NQ], f32, tag="row", bufs=3, name="yrow")
                    for half in range(2):
                        yp = accp.tile([128, 512], f32, tag="acc", name="yp")
                        for hd in range(NH):
                            nc.tensor.matmul(
                                yp, band[:, hd, :],
                                attnT[:, hd, half * 512:(half + 1) * 512],
                                start=(hd == 0), stop=(hd == NH - 1),
                            )
                        nc.scalar.activation(
                            yrow[:, half * 512:(half + 1) * 512], yp, AF.Identity,
                            bias=bo_sb[:, m:m + 1],
                        )
                    xq_t = rowp.tile([128, NQ], f32, tag="row", bufs=3, name="xq_t")
                    nc.sync.dma_start(xq_t, xqT_d[m * 128:(m + 1) * 128, :])
                    nc.vector.tensor_add(yrow, yrow, xq_t)
                    nc.sync.dma_start(ydram[m * 128:(m + 1) * 128, :], yrow)
                    # LN2 stats: mean g at partition 32g, var g at partition 64+32g
                    for g in range(2):
                        sq2 = workp.tile([128, 512], bf16, tag="sq", bufs=3, name="sq2")
                        nc.scalar.activation(
                            sq2, yrow[:, g * 512:(g + 1) * 512], AF.Square
                        )
                        nc.tensor.matmul(
                            stat2[32 * g:32 * g + 1, :], ones_f,
                            yrow[:, g * 512:(g + 1) * 512],
                            start=(m == 0 and g == 0), stop=(m == HT - 1 and g == 1),
                            skip_group_check=True, tile_position=(0, 32 * g),
                        )
                        nc.tensor.matmul(
                            stat2[64 + 32 * g:64 + 32 * g + 1, :], ones_bf, sq2,
                            start=False, stop=(m == HT - 1 and g == 1),
                            skip_group_check=True, tile_position=(0, 64 + 32 * g),
                        )

            # ---- LN2 smalls + apply -> ln2T ----
            with nc.named_scope("ln2"):
                mbs2, rbs2 = [], []
                for g in range(2):
                    mb, rb = _ln_stats_smalls(
                        nc, srowp, bcp, mmp, ones1_bf, eps_t,
                        stat2[32 * g:32 * g + 1, :],
                        stat2[64 + 32 * g:64 + 32 * g + 1, :], 1.0 / H,
                    )
                    mbs2.append(mb)
                    rbs2.append(rb)
                ln2T = b32p.tile([128, KT, NQ], bf16, tag="t32", name="ln2T")
                for k in range(KT):
                    yk = rowp.tile([128, NQ], f32, tag="row", bufs=3, name="yk")
                    nc.sync.dma_start(yk, ydram[k * 128:(k + 1) * 128, :])
                    for g in range(2):
                        t1 = workp.tile([128, 512], bf16, tag="t1", bufs=3, name="t1b")
                        nc.vector.tensor_sub(t1, yk[:, g * 512:(g + 1) * 512], mbs2[g])
                        nc.vector.tensor_mul(
                            ln2T[:, k, g * 512:(g + 1) * 512], t1, rbs2[g]
                        )

            # ---- MLP in two token halves (gT half fits the 64KB/p slot) ----
            with nc.named_scope("mlp"):
                for half in range(2):
                    tok = half * 512
                    gTh = bigp.tile([128, IT, 512], bf16, tag="t64", name="gTh")
                    for mi in range(IT):
                        band = bandp.tile(
                            [128, HT, 128], bf16, tag="band", bufs=3, name="bandf"
                        )
                        nc.sync.dma_start(band, wfc_r[mi])
                        fp = accp.tile([128, 512], f32, tag="acc", name="fp")
                        for k in range(KT):
                            nc.tensor.matmul(
                                fp, band[:, k, :], ln2T[:, k, tok:tok + 512],
                                start=(k == 0), stop=(k == KT - 1),
                            )
                        nc.scalar.activation(
                            gTh[:, mi, :], fp, AF.Gelu_apprx_tanh,
                            bias=bfc_sb[:, mi:mi + 1],
                        )
                    for m in range(HT):
                        pj = accp.tile([128, 512], f32, tag="acc", name="pj")
                        for kg in range(4):
                            band = bandp.tile(
                                [128, HT, 128], bf16, tag="band", bufs=3, name="bandp"
                            )
                            nc.sync.dma_start(
                                band, wproj_r[m, :, kg * 16:(kg + 1) * 16, :]
                            )
                            for kk in range(HT):
                                k = kg * 16 + kk
                                nc.tensor.matmul(
                                    pj, band[:, kk, :], gTh[:, k, :],
                                    start=(k == 0), stop=(k == IT - 1),
                                )
                        ot = rowp.tile([128, 512], f32, tag="row", bufs=3, name="ot")
                        nc.scalar.activation(
                            ot, pj, AF.Identity, bias=bproj_sb[:, m:m + 1]
                        )
                        yk2 = rowp.tile([128, 512], f32, tag="row", bufs=3, name="yk2")
                        nc.sync.dma_start(
                            yk2, ydram[m * 128:(m + 1) * 128, tok:tok + 512]
                        )
                        nc.vector.tensor_add(ot, ot, yk2)
                        nc.sync.dma_start(
                            out_d[m * 128:(m + 1) * 128, tok:tok + 512], ot
                        )
    _split_excess_waits(nc)
    return nc


_PROG = None


def _get_prog():
    global _PROG
    if _PROG is None:
        _PROG = _build_program()
    return _PROG


def kernel(hidden_states, ln1_g, ln1_b, ln2_g, ln2_b, wq, bq, wkv, bkv,
           wo, bo, wfc, bfc, wproj, bproj):
    hs = np.asarray(hidden_states, np.float32)
    ln1_g = np.asarray(ln1_g, np.float32)
    ln1_b = np.asarray(ln1_b, np.float32)
    ln2_g = np.asarray(ln2_g, np.float32)
    ln2_b = np.asarray(ln2_b, np.float32)
    wq = np.asarray(wq, np.float32)
    wkv = np.asarray(wkv, np.float32)
    wo = np.asarray(wo, np.float32)
    wfc = np.asarray(wfc, np.float32)
    wproj = np.asarray(wproj, np.float32)

    # Fold LN gains/biases into the following matmuls; fold qk scale into K.
    wq_e = ln1_g[:, None] * wq
    bq_e = np.asarray(bq, np.float32) + ln1_b @ wq
    wkv_e = (ln1_g[:, None] * wkv).copy()
    bkv_e = (np.asarray(bkv, np.float32) + ln1_b @ wkv).copy()
    scale = 1.0 / np.sqrt(D)
    wkv_e[:, :D] *= scale
    bkv_e[:D] *= scale
    wfc_e = ln2_g[:, None] * wfc
    bfc_e = np.asarray(bfc, np.float32) + ln2_b @ wfc

    def to_bf(a):
        return np.ascontiguousarray(a.astype(ml_dtypes.bfloat16))

    def tile_mk(w, mt, kt):
        # [K, M] -> [(m p), (k n)] with arr[m*128+p, k*128+n] = w[k*128+p, m*128+n]
        return np.ascontiguousarray(
            w.reshape(kt, 128, mt, 128).transpose(2, 1, 0, 3).reshape(mt * 128, kt * 128)
        )

    wq_t = to_bf(tile_mk(wq_e, HT, HT))
    wo_t = to_bf(tile_mk(wo, HT, HT))
    wfc_t = to_bf(tile_mk(wfc_e, IT, HT))
    wproj_t = to_bf(tile_mk(wproj, HT, IT))
    # [p, (k n)] layouts for the KV weight columns
    wkvK_t = to_bf(wkv_e[:, :D].reshape(KT, 128, D).transpose(1, 0, 2).reshape(128, H))
    wkvV_t = to_bf(wkv_e[:, D:].reshape(KT, 128, D).transpose(1, 0, 2).reshape(128, H))

    bq_c = np.ascontiguousarray(bq_e)
    bo_c = np.ascontiguousarray(np.asarray(bo, np.float32))
    bfc_c = np.ascontiguousarray(bfc_e)
    bproj_c = np.ascontiguousarray(np.asarray(bproj, np.float32))
    bkK_c = np.ascontiguousarray(bkv_e[:D])
    bkV_c = np.ascontiguousarray(bkv_e[D:].astype(ml_dtypes.bfloat16))

    in_maps = []
    perms = []
    for c in range(8):
        b, h = divmod(c, 2)
        # permuted global block order: own interleaved blocks first
        own = [2 * i + h for i in range(8)]
        other = [2 * i + (1 - h) for i in range(8)]
        blocks = own + other
        perms.append(blocks)
        cols = np.concatenate([np.arange(g * 128, (g + 1) * 128) for g in blocks])
        xTb = hs[b].T  # [H, S]
        xT_p = np.ascontiguousarray(xTb[:, cols].astype(ml_dtypes.bfloat16))
        xqT = np.ascontiguousarray(xTb[:, cols[:NQ]].astype(np.float32))
        # global token index of each permuted column / k-tile row
        gidx = cols.astype(np.float32)
        qgb = np.ascontiguousarray(
            np.broadcast_to(gidx[:NQ].reshape(2, 512)[:, None, :], (2, 128, 512))
            .reshape(256, 512)
        )
        kvec = np.ascontiguousarray(gidx.reshape(KT, 128).T)  # [128, KT]
        in_maps.append(dict(
            xT=xT_p, xqT=xqT, wq=wq_t, wo=wo_t, wfc=wfc_t, wproj=wproj_t,
            wkvK=wkvK_t, wkvV=wkvV_t, bq=bq_c, bo=bo_c, bfc=bfc_c,
            bproj=bproj_c, bkK=bkK_c, bkV=bkV_c, qgb=qgb, kvec=kvec,
        ))

    res = run_bass_kernel_spmd(_get_prog(), in_maps, core_ids=list(range(8)))
    kernel.last_result = res

    out = np.empty((B, S, H), np.float32)
    for c in range(8):
        b, h = divmod(c, 2)
        outT = np.asarray(res.results[c]["outT"])  # [H, NQ]
        blocks = perms[c][:8]
        o = outT.T.reshape(8, 128, H)
        for i, g in enumerate(blocks):
            out[b, g * 128:(g + 1) * 128, :] = o[i]
    return out


kernel.last_result = None
